# revision 19
# baseline (speedup 1.0000x reference)
"""Trainium2 Bass kernel for nn_MemoryAugmentedModel (gnn_message_passing).

Math: the reference only consumes row N-1 of the GAT output, so the dense
[N,N,H] attention collapses:
  out[-1] = (1/H) * sum_h gat_w_h @ (softmax_j(lrelu(a_dst[-1,h]+a_src[j,h])) @ nf) + gat_bias
with a_src = nf @ V_src^T, V_src[h] = att_src[h] @ gat_w_h  (same for dst).
Then LayerNorm -> proj/LoRA offset -> embedding gather with offset added to
each sequence's first token.

Sharding (8 cores): gat_w / node_features split by input-feature columns
(e-chunks of 256) -> partial src logits (+dst-last ride) AllReduce #1 (bf16,
[128,68]) -> replicated softmax -> per-core agg over its e-chunk -> partial
out[-1] row AllReduce #2 (bf16 [1,2048], gat_bias folded in via a K=1
matmul) -> replicated LN stats; LN is algebraically folded into the
proj/LoRA offset (host-precomputed G/CONST rows) -> per-core offset chunk
AllGather #3 -> each core gathers 1024 of the 8192 output rows from a bf16
embedding table; a tiny [8,256] re-gather of each core's first row lets the
masked offset add run on 8 partitions.

Latency structure: a tiny warmup AllGather is the first gpsimd
instruction, so the CC barrier (~35-50us of launch-skew rendezvous) starts
immediately and the first real collective pays no init. Collective payload
hops ride the scalar queue after its short early bulk burst; attention-path
loads go first on sync (+2 w_nat chunks on scalar); the embedding gather
and out_sl writes ride gpsimd/sync so all bulk DMA drains before the
post-AR1 hops. Dummy matmul chains span the AR1/AR2 waits to hold the PE
at full clock for the agg/out-pass/proj matvecs. Activation tables are
preloaded Sqrt-then-Exp so the softmax Exp runs hot.
"""

import os
import sys
import types

import numpy as np

NCORES = 8
N = 2048
D = 2048
H = 4
R = 32
V = 32000
B = 4
S = 2048

EC = D // NCORES          # 256: e-columns (input features) per core
FC = D // NCORES          # 256: offset rows per core
ROWS = (B * S) // NCORES  # 1024: output embedding rows per core
NG = ROWS // 128          # 8 gather groups per core
NU = D // 128             # 16: 128-row chunks of a length-D axis
NT = (H * D) // 128       # 64: 128-row strips of gat_w

_CACHE = {}


def _install_ntff_shim():
    """Register the axon NTFF profile hook missing from this image's antenv."""
    if "antenv.axon_hooks" in sys.modules:
        return
    try:
        import antenv
        from trn_agent_boot.trn_boot import _ntff_profile_via_ctypes
    except Exception:
        return
    mod = types.ModuleType("antenv.axon_hooks")
    mod._hook = None
    mod.set_axon_ntff_profile_hook = lambda h: setattr(mod, "_hook", h)
    mod.get_axon_ntff_profile_hook = lambda: mod._hook
    sys.modules["antenv.axon_hooks"] = mod
    antenv.axon_hooks = mod
    try:
        mod.set_axon_ntff_profile_hook(
            _ntff_profile_via_ctypes("/opt/axon/libaxon_pjrt.so")
        )
    except Exception:
        pass


def _build():
    import concourse.bacc as bacc
    import concourse.bass as bass
    import concourse.tile as tile
    from concourse import mybir

    f32 = mybir.dt.float32
    bf16 = mybir.dt.bfloat16
    i32 = mybir.dt.int32
    RG = [list(range(NCORES))]
    AT = mybir.AluOpType
    AF = mybir.ActivationFunctionType

    nc = bacc.Bacc("TRN2", target_bir_lowering=False, debug=False,
                   num_devices=NCORES)

    din = lambda name, shape, dt: nc.dram_tensor(name, shape, dt, kind="ExternalInput").ap()
    att_st = din("att_st", [128, NT, 2 * H], bf16)   # zero-padded per strip
    w_nat = din("w_nat", [128, NT, EC], bf16)
    nf_tr = din("nf_tr", [2 * 128, N], bf16)
    nf_pre = din("nf_pre", [128, NU, EC], bf16)
    w_tr = din("w_tr", [2 * 128, H * D], bf16)
    proj_pre = din("proj_pre", [128, NU, FC], bf16)
    lora_at = din("lora_at", [128, NU, R], bf16)
    lora_bt = din("lora_bt", [R, FC], bf16)
    gb_row = din("gb_row", [1, D], bf16)
    gamma_r = din("gamma_r", [128, NU], f32)
    g_row = din("g_row", [1, FC], f32)
    c_row = din("c_row", [1, FC], f32)
    ids_r = din("ids_r", [128, NG], i32)
    ids8 = din("ids8", [8, 1], i32)
    mask8 = din("mask8", [8, 1], f32)
    embed = din("embed", [V, D], bf16)

    out_sl = nc.dram_tensor("out_sl", [ROWS, D], bf16, kind="ExternalOutput").ap()

    dshared = lambda name, shape, dt: nc.dram_tensor(
        name, shape, dt, kind="Internal", addr_space="Shared").ap()
    dlocal = lambda name, shape, dt: nc.dram_tensor(
        name, shape, dt, kind="Internal").ap()
    wu_in = dlocal("wu_in", [1, 1], f32)
    wu_out = dlocal("wu_out", [2, 1], f32)
    ar1_in = dlocal("ar1_in", [128, 68], bf16)
    ar1_out = dshared("ar1_out", [128, 68], bf16)
    ar2_in = dlocal("ar2_in", [1, D], bf16)
    ar2_out = dshared("ar2_out", [1, D], bf16)
    ag3_in = dlocal("ag3_in", [1, FC], bf16)
    ag3_out = dshared("ag3_out", [NCORES, FC], bf16)

    with tile.TileContext(nc) as tc:
        import contextlib
        ctx = contextlib.ExitStack()
        with ctx:
            const = ctx.enter_context(tc.tile_pool(name="const", bufs=1))
            embp = ctx.enter_context(tc.tile_pool(name="embp", bufs=NG))

            # ---- warmup AllGather: the barrier starts when gpsimd reaches
            # the first collective instruction, so code it first; the tiny
            # memset->DMA producer chain pins it early in the schedule.
            wu_sb = const.tile([1, 1], f32)
            nc.vector.memset(wu_sb[:], 0.0)
            nc.gpsimd.dma_start(wu_in[:], wu_sb[:])
            nc.gpsimd.collective_compute(
                "AllGather", AT.bypass,
                replica_groups=[[2 * i, 2 * i + 1] for i in range(NCORES // 2)],
                ins=[wu_in[:].opt()], outs=[wu_out[:].opt()])

            # ---- tiny const tiles + act-table preloads (Sqrt then Exp, so
            # the softmax Exp finds a hot table; chained to force order) ----
            eps_sb = const.tile([1, 1], f32)
            nc.vector.memset(eps_sb[:], 1e-5)
            dum_sb = const.tile([1, 1], f32)
            nc.scalar.activation(out=dum_sb[:], in_=eps_sb[:], func=AF.Sqrt)
            dum2_sb = const.tile([1, 1], f32)
            nc.scalar.activation(out=dum2_sb[:], in_=dum_sb[:], func=AF.Exp)
            # ---- index loads (sync) ---------------------------------------
            ids_sb = const.tile([128, NG], i32)
            nc.sync.dma_start(ids_sb[:], ids_r[:])
            ids8_sb = const.tile([8, 1], i32)
            nc.sync.dma_start(ids8_sb[:], ids8[:])

            ones1b = const.tile([1, 128], bf16)
            nc.vector.memset(ones1b[:], 1.0)
            oneb = const.tile([1, 1], bf16)
            nc.vector.memset(oneb[:], 1.0)
            onescf = const.tile([128, 1], f32)
            nc.vector.memset(onescf[:], 1.0)
            ident_sb = const.tile([128, 128], bf16)
            from concourse.masks import make_identity
            make_identity(nc, ident_sb[:])

            # ---- embedding gathers (gpsimd): mini row-0 gather first ------
            emb0_sb = const.tile([8, EC], bf16)
            emb_r8 = embed[:, :].rearrange("v (s f) -> (v s) f", f=EC)
            nc.gpsimd.indirect_dma_start(
                out=emb0_sb[:], out_offset=None, in_=emb_r8,
                in_offset=bass.IndirectOffsetOnAxis(ap=ids8_sb[:, 0:1], axis=0),
            )
            emb_tiles = []
            for g in range(NG):
                et = embp.tile([128, D], bf16, name=f"emb{g}", tag="emb")
                nc.gpsimd.indirect_dma_start(
                    out=et[:], out_offset=None, in_=embed[:, :],
                    in_offset=bass.IndirectOffsetOnAxis(ap=ids_sb[:, g:g + 1], axis=0),
                )
                emb_tiles.append(et)

            # ---- attention-path loads (sync: pre-AR1 critical) ------------
            # scalar gets a short bulk burst then becomes the latency-hop
            # queue; w_tr/nf_pre/params/writes go on sync after ar1_in.
            attst_sb = const.tile([128, NT, 2 * H], bf16)
            nc.sync.dma_start(attst_sb[:], att_st[:])
            wn_sb = const.tile([128, NT, EC], bf16)
            for ch in range(4):
                eng = nc.sync if ch < 2 else nc.scalar
                eng.dma_start(wn_sb[:, ch * 16:(ch + 1) * 16, :],
                              w_nat[:, ch * 16:(ch + 1) * 16, :])
            nft_sb = []
            for half in range(2):
                t = const.tile([128, N], bf16, name=f"nft{half}", tag=f"nft{half}")
                nc.sync.dma_start(t[:], nf_tr[half * 128:(half + 1) * 128, :])
                nft_sb.append(t)
            proj_sb = const.tile([128, NU, FC], bf16)
            nc.scalar.dma_start(proj_sb[:], proj_pre[:])
            lat_sb = const.tile([128, NU, R], bf16)
            nc.scalar.dma_start(lat_sb[:], lora_at[:])
            lbt_sb = const.tile([R, FC], bf16)
            nc.scalar.dma_start(lbt_sb[:], lora_bt[:])

            # ---- phase 1: V = att @ W -> vT; a partials + dst-last ride ---
            vsb = const.tile([2 * H, EC], bf16)
            vT_sb = [const.tile([128, 2 * H], bf16, name=f"vT{i}", tag=f"vT{i}")
                     for i in range(2)]
            a_loc = const.tile([128, 68], bf16)
            with tc.tile_pool(name="pp1", bufs=1, space="PSUM") as pp1, \
                 tc.tile_pool(name="pp1t", bufs=2, space="PSUM") as pp1t:
                ps_v2 = pp1.tile([2 * H, EC], f32)
                for t in range(NT):
                    nc.tensor.matmul(out=ps_v2[:], lhsT=attst_sb[:, t, :],
                                     rhs=wn_sb[:, t, :],
                                     start=(t == 0), stop=(t == NT - 1))
                nc.vector.tensor_copy(out=vsb[:], in_=ps_v2[:])
                for half in range(2):
                    ps_t = pp1t.tile([128, 2 * H], bf16, tag="pst")
                    nc.tensor.transpose(out=ps_t[:],
                                        in_=vsb[:, half * 128:(half + 1) * 128],
                                        identity=ident_sb[0:2 * H, 0:2 * H])
                    nc.vector.tensor_copy(out=vT_sb[half][:], in_=ps_t[:])
                # a_src[j, h] partials: j = jc*128 + m
                ps_a = pp1.tile([128, 64], f32)
                for jc in range(NU):
                    for half in range(2):
                        nc.tensor.matmul(
                            out=ps_a[:, jc * 4:(jc + 1) * 4],
                            lhsT=nft_sb[half][:, jc * 128:(jc + 1) * 128],
                            rhs=vT_sb[half][:, 0:H],
                            start=(half == 0), stop=(half == 1))
                # dst-last ride: a_dst[N-1, h] partial
                ps_d = pp1.tile([1, H], f32)
                for half in range(2):
                    nc.tensor.matmul(
                        out=ps_d[:], lhsT=nft_sb[half][:, N - 1:N],
                        rhs=vT_sb[half][:, H:2 * H],
                        start=(half == 0), stop=(half == 1))
                nc.vector.tensor_copy(out=a_loc[:, 0:64], in_=ps_a[:])
                nc.vector.memset(a_loc[:, 64:68], 0.0)
                nc.vector.tensor_copy(out=a_loc[0:1, 64:68], in_=ps_d[:])
            nc.gpsimd.dma_start(ar1_in[:], a_loc[:])
            nc.gpsimd.collective_compute(
                "AllReduce", AT.add, replica_groups=RG,
                ins=[ar1_in[:].opt()], outs=[ar1_out[:].opt()])

            # ---- late bulk on sync: needed from ~AR1-end onward -----------
            nf_sb = const.tile([128, NU, EC + 1], bf16)
            nc.sync.dma_start(nf_sb[:, :, 0:EC], nf_pre[:])
            nc.vector.memset(nf_sb[:, :, EC:EC + 1], 1.0)
            gb_sb = const.tile([1, D], bf16)
            nc.sync.dma_start(gb_sb[:], gb_row[:])
            gamma_sb = const.tile([128, NU], f32)
            nc.sync.dma_start(gamma_sb[:], gamma_r[:])
            g_sb = const.tile([1, FC], f32)
            nc.sync.dma_start(g_sb[:], g_row[:])
            c_sb = const.tile([1, FC], f32)
            nc.sync.dma_start(c_sb[:], c_row[:])
            mask8_sb = const.tile([8, 1], f32)
            nc.sync.dma_start(mask8_sb[:], mask8[:])
            wt_sb = []
            for half in range(2):
                t = const.tile([128, H * D], bf16, name=f"wt{half}", tag=f"wt{half}")
                nc.sync.dma_start(t[:], w_tr[half * 128:(half + 1) * 128, :])
                wt_sb.append(t)
            # out_sl bulk writes split across the gpsimd and sync queues so
            # all bulk DMA drains by ~75us and cannot alias the post-AR1
            # latency-hop semaphores.
            for g in range(NG):
                eng = nc.gpsimd if g % 2 == 0 else nc.sync
                if g == 0:
                    eng.dma_start(out_sl[1:128, :], emb_tiles[0][1:128, :])
                else:
                    eng.dma_start(out_sl[g * 128:(g + 1) * 128, :],
                                  emb_tiles[g][:])

            # ---- keep the PE clock ramped through the AR1 wait ------------
            with tc.tile_pool(name="ppw1", bufs=1, space="PSUM") as ppw1:
                ps_w1 = ppw1.tile([2 * H, EC], f32)
                for i in range(200):
                    nc.tensor.matmul(out=ps_w1[:], lhsT=attst_sb[:, i % NT, :],
                                     rhs=wn_sb[:, i % NT, :],
                                     start=True, stop=True)

            # ---- softmax weights (replicated) -----------------------------
            a_sb = const.tile([128, 68], bf16)
            nc.scalar.dma_start(a_sb[:], ar1_out[:])
            wu_exp = const.tile([128, NU, H], bf16)
            with tc.tile_pool(name="ppd", bufs=1, space="PSUM") as ppd:
                ps_db = ppd.tile([128, H], f32)
                nc.tensor.matmul(out=ps_db[:], lhsT=ones1b[:],
                                 rhs=a_sb[0:1, 64:68], start=True, stop=True)
                dstb_sb = const.tile([128, H], f32)
                nc.vector.tensor_copy(out=dstb_sb[:], in_=ps_db[:])
            dstb_b = bass.AP(tensor=dstb_sb[:].tensor, offset=dstb_sb[:].offset,
                             ap=[dstb_sb[:].ap[0], [0, NU], [1, H]])
            a_srcv = a_sb[:, 0:64].rearrange("p (u c) -> p u c", c=H)
            l_sb = const.tile([128, NU, H], f32)
            nc.vector.tensor_tensor(out=l_sb[:], in0=a_srcv, in1=dstb_b, op=AT.add)
            l2_sb = const.tile([128, NU, H], f32)
            nc.vector.tensor_scalar_mul(l2_sb[:], l_sb[:], 0.2)
            nc.vector.tensor_tensor(out=l_sb[:], in0=l_sb[:], in1=l2_sb[:], op=AT.max)
            nc.scalar.activation(out=wu_exp[:], in_=l_sb[:], func=AF.Exp)

            # ---- agg = attnU^T @ [nf | 1]; normalize; transpose -----------
            aggT_sb = [const.tile([128, H], bf16, name=f"aggT{i}", tag=f"aggT{i}")
                       for i in range(2)]
            with tc.tile_pool(name="ppg", bufs=1, space="PSUM") as ppg, \
                 tc.tile_pool(name="ppab", bufs=2, space="PSUM") as ppab:
                ps_agg = ppg.tile([H, EC + 1], f32)
                for u in range(NU):
                    nc.tensor.matmul(
                        out=ps_agg[:], lhsT=wu_exp[:, u, :], rhs=nf_sb[:, u, :],
                        start=(u == 0), stop=(u == NU - 1))
                rz_sb = const.tile([H, 1], f32)
                nc.vector.reciprocal(out=rz_sb[:], in_=ps_agg[:, EC:EC + 1])
                nc.vector.tensor_scalar_mul(rz_sb[:], rz_sb[:], 1.0 / H)
                aggn_sb = const.tile([H, EC], bf16)
                nc.vector.tensor_scalar_mul(aggn_sb[:], ps_agg[:, 0:EC], rz_sb[:])
                for half in range(2):
                    ps_gt = ppab.tile([128, H], bf16, tag="psgt")
                    nc.tensor.transpose(out=ps_gt[:],
                                        in_=aggn_sb[:, half * 128:(half + 1) * 128],
                                        identity=ident_sb[0:H, 0:H])
                    nc.vector.tensor_copy(out=aggT_sb[half][:], in_=ps_gt[:])

            # ---- out[-1] partial row [1, 2048], gat_bias folded in --------
            row_loc = const.tile([1, D], bf16)
            with tc.tile_pool(name="ppo", bufs=1, space="PSUM") as ppo:
                for q in range(4):
                    ps_o = ppo.tile([1, 512], f32, name=f"pso{q}", tag=f"pso{q}")
                    for h in range(H):
                        for half in range(2):
                            nc.tensor.matmul(
                                out=ps_o[:],
                                lhsT=aggT_sb[half][:, h:h + 1],
                                rhs=wt_sb[half][:, h * D + q * 512:h * D + (q + 1) * 512],
                                start=(h == 0 and half == 0), stop=False)
                    nc.tensor.matmul(
                        out=ps_o[:], lhsT=oneb[:],
                        rhs=gb_sb[0:1, q * 512:(q + 1) * 512],
                        start=False, stop=True)
                    if q < 2:
                        nc.vector.tensor_copy(
                            out=row_loc[:, q * 512:(q + 1) * 512], in_=ps_o[:])
                    else:
                        nc.scalar.activation(
                            out=row_loc[:, q * 512:(q + 1) * 512], in_=ps_o[:],
                            func=AF.Copy)
            nc.scalar.dma_start(ar2_in[:], row_loc[:])
            # keep the PE pstate ramped through the AR2 wait so the proj
            # matvec runs at full clock (results unused)
            with tc.tile_pool(name="ppw", bufs=1, space="PSUM") as ppw:
                ps_w = ppw.tile([1, 512], f32)
                for i in range(56):
                    nc.tensor.matmul(out=ps_w[:], lhsT=aggT_sb[0][:, 0:1],
                                     rhs=wt_sb[0][:, 0:512],
                                     start=True, stop=True)
            nc.gpsimd.collective_compute(
                "AllReduce", AT.add, replica_groups=RG,
                ins=[ar2_in[:].opt()], outs=[ar2_out[:].opt()])

            # ---- LN stats from [128, 16] view; LN folded into offset ------
            x_sb = const.tile([128, NU], bf16)
            nc.scalar.dma_start(
                x_sb[:], ar2_out[:].rearrange("r (p u) -> (r p) u", u=NU))
            xx_sb = const.tile([128, NU], f32)
            nc.vector.tensor_tensor(out=xx_sb[:], in0=x_sb[:], in1=x_sb[:],
                                    op=AT.mult)
            xs2_sb = const.tile([128, 2], f32)
            nc.vector.reduce_sum(out=xs2_sb[:, 0:1], in_=x_sb[:],
                                 axis=mybir.AxisListType.X)
            nc.vector.reduce_sum(out=xs2_sb[:, 1:2], in_=xx_sb[:],
                                 axis=mybir.AxisListType.X)
            stats_sb = const.tile([1, 2], f32)
            with tc.tile_pool(name="pps", bufs=1, space="PSUM") as pps:
                ps_s = pps.tile([1, 2], f32)
                nc.tensor.matmul(out=ps_s[:], lhsT=onescf[:], rhs=xs2_sb[:],
                                 start=True, stop=True)
                nc.vector.tensor_copy(out=stats_sb[:], in_=ps_s[:])
            st2_sb = const.tile([1, 2], f32)
            nc.vector.tensor_scalar_mul(st2_sb[:], stats_sb[:], 1.0 / D)
            mu_sb = st2_sb[:, 0:1]
            var_sb = const.tile([1, 1], f32)
            mu2_sb = const.tile([1, 1], f32)
            nc.vector.tensor_tensor(out=mu2_sb[:], in0=mu_sb, in1=mu_sb,
                                    op=AT.mult)
            nc.vector.tensor_tensor(out=var_sb[:], in0=st2_sb[:, 1:2],
                                    in1=mu2_sb[:], op=AT.subtract)
            sd_sb = const.tile([1, 1], f32)
            nc.scalar.activation(out=sd_sb[:], in_=var_sb[:], func=AF.Sqrt,
                                 bias=eps_sb[:], scale=1.0)
            rstd_sb = const.tile([1, 1], f32)
            nc.vector.reciprocal(out=rstd_sb[:], in_=sd_sb[:])
            rmu_sb = const.tile([1, 1], f32)
            nc.vector.tensor_tensor(out=rmu_sb[:], in0=rstd_sb[:], in1=mu_sb,
                                    op=AT.mult)
            u_sb = const.tile([128, NU], bf16)
            nc.vector.tensor_tensor(out=u_sb[:], in0=x_sb[:], in1=gamma_sb[:],
                                    op=AT.mult)

            # ---- offset chunk: rstd*(P@u + LS*B@(A@u)) - rmu*G + C --------
            off_sb = const.tile([1, FC], bf16)
            with tc.tile_pool(name="ppp", bufs=2, space="PSUM") as ppp:
                ps_t2 = ppp.tile([1, R], f32, tag="lt")
                for u in range(NU):
                    nc.tensor.matmul(out=ps_t2[:], lhsT=u_sb[:, u:u + 1],
                                     rhs=lat_sb[:, u, :],
                                     start=(u == 0), stop=(u == NU - 1))
                lt_row = const.tile([1, R], bf16)
                nc.vector.tensor_scalar_mul(lt_row[:], ps_t2[:], 2.0)  # alpha/r
                ps_tt = ppp.tile([R, 1], bf16, tag="ltT")
                nc.tensor.transpose(out=ps_tt[:], in_=lt_row[:],
                                    identity=ident_sb[0:1, 0:1])
                ltT_sb = const.tile([R, 1], bf16)
                nc.vector.tensor_copy(out=ltT_sb[:], in_=ps_tt[:])
                ps_pj = ppp.tile([1, FC], f32, tag="pj")
                for u in range(NU):
                    nc.tensor.matmul(
                        out=ps_pj[:], lhsT=u_sb[:, u:u + 1],
                        rhs=proj_sb[:, u, :], start=(u == 0), stop=False)
                nc.tensor.matmul(out=ps_pj[:], lhsT=ltT_sb[:], rhs=lbt_sb[:],
                                 start=False, stop=True)
                dg_sb = const.tile([1, FC], f32)
                nc.vector.tensor_scalar_mul(dg_sb[:], g_sb[:], rmu_sb[:])
                e_sb = const.tile([1, FC], f32)
                nc.vector.tensor_tensor(out=e_sb[:], in0=c_sb[:], in1=dg_sb[:],
                                        op=AT.subtract)
                nc.vector.tensor_scalar_mul(off_sb[:], ps_pj[:], rstd_sb[:])
                nc.vector.tensor_tensor(out=off_sb[:], in0=off_sb[:], in1=e_sb[:],
                                        op=AT.add)
            nc.scalar.dma_start(ag3_in[:], off_sb[:])
            nc.gpsimd.collective_compute(
                "AllGather", AT.bypass, replica_groups=RG,
                ins=[ag3_in[:].opt()], outs=[ag3_out[:].opt()])

            # ---- first-token row: masked offset add on 8 partitions -------
            off8_sb = const.tile([8, FC], bf16)
            nc.scalar.dma_start(off8_sb[:], ag3_out[:])
            t8_sb = const.tile([8, FC], f32)
            nc.vector.tensor_scalar_mul(t8_sb[:], off8_sb[:], mask8_sb[:])
            out0_sb = const.tile([8, FC], bf16)
            nc.vector.tensor_tensor(out=out0_sb[:], in0=emb0_sb[:], in1=t8_sb[:],
                                    op=AT.add)
            nc.scalar.dma_start(
                out_sl[0:1, :].rearrange("r (s f) -> (r s) f", f=EC), out0_sb[:])

    nc.compile()
    return nc


def _prep_inputs(inputs):
    import ml_dtypes
    bf16 = ml_dtypes.bfloat16

    nf = np.asarray(inputs["node_features"], dtype=np.float32)
    ids = np.asarray(inputs["input_ids"], dtype=np.int32).reshape(-1)
    gw = np.asarray(inputs["gat_w"], dtype=np.float32)
    att_src = np.asarray(inputs["att_src"], dtype=np.float32)
    att_dst = np.asarray(inputs["att_dst"], dtype=np.float32)
    gbias = np.asarray(inputs["gat_bias"], dtype=np.float32)
    gamma = np.asarray(inputs["ln_gamma"], dtype=np.float32)
    beta = np.asarray(inputs["ln_beta"], dtype=np.float32)
    pw = np.asarray(inputs["proj_w"], dtype=np.float32)
    pb = np.asarray(inputs["proj_b"], dtype=np.float32)
    la = np.asarray(inputs["lora_a"], dtype=np.float32)
    lb = np.asarray(inputs["lora_b"], dtype=np.float32)
    emb_bf = np.ascontiguousarray(
        np.asarray(inputs["embed"], dtype=np.float32).astype(bf16))

    # LN folded into offset: G = P@gamma + LS*B@(A@gamma),
    # CONST = P@beta + LS*B@(A@beta) + pb
    pw64, lb64, la64 = pw.astype(np.float64), lb.astype(np.float64), la.astype(np.float64)
    g64, b64 = gamma.astype(np.float64), beta.astype(np.float64)
    G_full = (pw64 @ g64 + 2.0 * (lb64 @ (la64 @ g64))).astype(np.float32)
    C_full = (pw64 @ b64 + 2.0 * (lb64 @ (la64 @ b64)) + pb).astype(np.float32)

    att_strips = np.zeros((NT, 128, 2 * H), dtype=np.float32)
    for t in range(NT):
        h, u = t // NU, t % NU
        att_strips[t, :, h] = att_src[h, u * 128:(u + 1) * 128]
        att_strips[t, :, H + h] = att_dst[h, u * 128:(u + 1) * 128]
    att_st = np.ascontiguousarray(
        att_strips.transpose(1, 0, 2).astype(bf16))  # [128, NT, 2H]
    lora_at = np.ascontiguousarray(la.T.reshape(128, NU, R).astype(bf16))
    gamma_r = np.ascontiguousarray(gamma.reshape(128, NU))

    in_maps = []
    for c in range(NCORES):
        ech = slice(c * EC, (c + 1) * EC)
        fch = slice(c * FC, (c + 1) * FC)
        w_sl = gw[:, ech]
        nf_sl = nf[:, ech]
        m = {
            "att_st": att_st,
            "w_nat": np.ascontiguousarray(
                w_sl.reshape(NT, 128, EC).transpose(1, 0, 2).astype(bf16)),
            "w_tr": np.ascontiguousarray(w_sl.T.astype(bf16)),
            "nf_tr": np.ascontiguousarray(nf_sl.T.astype(bf16)),
            "nf_pre": np.ascontiguousarray(
                nf_sl.reshape(NU, 128, EC).transpose(1, 0, 2).astype(bf16)),
            "proj_pre": np.ascontiguousarray(
                pw[fch, :].T.reshape(128, NU, FC).astype(bf16)),
            "lora_at": lora_at,
            "lora_bt": np.ascontiguousarray(lb[fch, :].T.astype(bf16)),
            "gb_row": (gbias.reshape(1, D).astype(bf16)
                       if c == 0 else np.zeros((1, D), dtype=bf16)),
            "gamma_r": gamma_r,
            "g_row": np.ascontiguousarray(G_full[fch].reshape(1, FC)),
            "c_row": np.ascontiguousarray(C_full[fch].reshape(1, FC)),
            "ids_r": np.ascontiguousarray(
                ids[c * ROWS:(c + 1) * ROWS].reshape(NG, 128).T),
            "ids8": np.ascontiguousarray(
                (ids[c * ROWS] * 8 + np.arange(8, dtype=np.int32))
                .reshape(8, 1).astype(np.int32)),
            "mask8": np.full((8, 1), 1.0 if c % 2 == 0 else 0.0,
                             dtype=np.float32),
            "embed": emb_bf,
        }
        in_maps.append(m)
    return in_maps


def kernel(**inputs):
    _install_ntff_shim()
    from concourse.bass_utils import run_bass_kernel_spmd

    if "nc" not in _CACHE:
        _CACHE["nc"] = _build()
    nc = _CACHE["nc"]

    in_maps = _prep_inputs(inputs)
    trace = bool(int(os.environ.get("KERNEL_TRACE", "0")))
    res = run_bass_kernel_spmd(nc, in_maps, core_ids=list(range(NCORES)),
                               trace=trace)
    if trace:
        _CACHE["last_result"] = res
        print(f"HW exec time: {res.exec_time_ns} ns", flush=True)

    out = np.concatenate([res.results[c]["out_sl"] for c in range(NCORES)], axis=0)
    return out.astype(np.float32).reshape(B, S, D)
